# revision 18
# baseline (speedup 1.0000x reference)
"""MIND loss on 8 Trainium2 NeuronCores (Bass/Tile), tap-based formulation.

Math (validated against the jax reference by a NumPy golden model): of the 80
neighbourhood shifts only those with |tx|,|ty| <= 1 act as real +-512-pixel
shifts (affine_grid semantics); the other 72 degenerate to blur(img2^2).  Per
128-row band only ONE of the +-512-row partners is in bounds, so each core
computes just 5 distinct response maps + Vimg:

  s0 = blur(o^2)                        weight 77 (72 + degenerate maps)
  f  = blur((o - p)^2)                  p = partner band (rows +-512)
  x  = blur((oL - oR)^2)                placed left (po) and right (mo)
  pu = blur((oL - pR)^2)  (left half),  mu = blur((oR - pL)^2) (right half)
  Vimg = (blur((o1-p1)^2 + 2*o1^2 + x1^2-in-both-halves))/4 + eps

This target executes ~1 instruction per ~30-100us regardless of operand size,
so the kernel is built to minimise INSTRUCTION COUNT (~57 total):

 - The host packs ALL map operand pairs side by side into two [128, 6192]
   bf16 blobs (IN1, IN2), with 3 guard columns between fields carrying the
   true neighbouring values, so ONE tensor_sub + ONE tensor_mul computes
   every squared field (s0, f, x, p, m, v1, o1^2, vx) at once.
 - x-conv (along the free dim) = 7 shifted-AP multiply-accumulate taps over
   the whole packed conv region (guards make field crossings correct).
 - y-conv (across partitions) = 6 partition-shifted SBUF->SBUF DMA copies of
   the x-conved tile (halo rows host-precomputed and DMA'd into the shifted
   gaps) + 7 multiply-accumulate taps.
 - Post: b = D2 * (-1/V) (6 muls), one big exp, max/num trees, rden =
   exp(-bmax), column-crop masks, and 3 fused multiply+reduce ops into
   per-column partials.  Row-crop (rows 7..1017) is resolved on the host by
   subtracting separately accumulated top7/bot6 partials.
"""

import sys
import numpy as np

sys.path.insert(0, "/opt/trn_rl_repo")

import ml_dtypes  # noqa: E402

BF = ml_dtypes.bfloat16

PATCH = 7
SIGMA = 2.0
EPS = 1e-5
H = W = 1024
NORM = 80.0 * 1011.0 * 1010.0

# ---- packed layout (free-dim offsets, bf16 elements) -----------------------
# every field: [gL(3) | interior | gR(3)]
_FIELDS = [("s0", 1024), ("f", 1024), ("x", 512), ("x2", 512), ("p", 512),
           ("m", 512), ("sv", 1024)]
_RAW = [("o1sq", 1024), ("vx", 512)]

FOFF = {}
_off = 0
for _n, _w in _FIELDS + _RAW:
    FOFF[_n] = (_off, _off + 3, _w)      # (block start, interior start, width)
    _off += _w + 6
WC = sum(w + 6 for _, w in _FIELDS)      # conv region width (4644)
WIN = _off                               # total packed width (6192)


def _g1d():
    ax = np.arange(PATCH, dtype=np.float64) - PATCH // 2
    return (np.exp(-(ax ** 2) / (2 * SIGMA ** 2)) /
            np.sqrt(2 * np.pi * SIGMA ** 2)).astype(np.float32)


G7 = _g1d()


# ---------------------------------------------------------------- host prep

def _band_rows(img, base, n):
    """rows base..base+n-1, zero-padded outside [0,1024)."""
    out = np.zeros((n, W), np.float32)
    lo, hi = max(0, base), min(H, base + n)
    if lo < hi:
        out[lo - base:hi - base] = img[lo:hi]
    return out


def _pack_pair(o2, p2, o1, p1, nrows):
    """Build IN1/IN2 [nrows, WIN] fp32 for one row-block (band or halo)."""
    A = np.zeros((nrows, WIN), np.float32)
    Bm = np.zeros((nrows, WIN), np.float32)
    L, R = slice(0, 512), slice(512, 1024)

    def put(name, a_int, b_int, agl=None, agr=None):
        _, i0, w = FOFF[name]
        A[:, i0:i0 + w] = a_int
        if b_int is not None:
            Bm[:, i0:i0 + w] = b_int
        if agl is not None:
            A[:, i0 - 3:i0] = agl
        if agr is not None:
            A[:, i0 + w:i0 + w + 3] = agr

    put("s0", o2, None)
    put("f", o2, p2)
    put("x", o2[:, L], o2[:, R], agr=o2[:, 512:515])
    put("x2", o2[:, L], o2[:, R], agl=o2[:, 509:512])
    put("p", o2[:, L], p2[:, R], agr=o2[:, 512:515])
    put("m", o2[:, R], p2[:, L], agl=o2[:, 509:512])
    put("sv", o1, p1)
    put("o1sq", o1, None)
    put("vx", o1[:, L], o1[:, R])
    return A, Bm


def _fold_sq(SQ):
    """Vimg folds on a squared-field array (in place): sv += 2*o1sq + vx."""
    svi = FOFF["sv"][1]
    oqi = FOFF["o1sq"][1]
    vxi = FOFF["vx"][1]
    SQ[:, svi:svi + 1024] += 2.0 * SQ[:, oqi:oqi + 1024]
    SQ[:, svi:svi + 512] += SQ[:, vxi:vxi + 512]
    SQ[:, svi + 512:svi + 1024] += SQ[:, vxi:vxi + 512]
    return SQ


def _host_halo_s1(img1, img2, c):
    """x-conved squared fields for the 6 halo rows -> HS1 [6, WC] fp32.

    Row order: 0-2 = rows r0-3..r0-1, 3-5 = rows r0+128..r0+130.
    """
    r0 = c * 128
    pbase = r0 + 512 if c < 4 else r0 - 512
    rows = [r0 - 3, r0 - 2, r0 - 1, r0 + 128, r0 + 129, r0 + 130]
    prow = [r - r0 + pbase for r in rows]
    o2 = np.concatenate([_band_rows(img2, r, 1) for r in rows])
    p2 = np.concatenate([_band_rows(img2, r, 1) for r in prow])
    o1 = np.concatenate([_band_rows(img1, r, 1) for r in rows])
    p1 = np.concatenate([_band_rows(img1, r, 1) for r in prow])
    A, Bm = _pack_pair(o2, p2, o1, p1, 6)
    A = A.astype(BF).astype(np.float32)
    Bm = Bm.astype(BF).astype(np.float32)
    SQ = _fold_sq((A - Bm) ** 2)
    S1 = np.zeros((6, WC), np.float32)
    for k in range(7):
        S1[:, 3:WC - 3] += G7[k] * SQ[:, k:k + WC - 6]
    return S1


def _core_inputs(img1, img2, c):
    r0 = c * 128
    pbase = r0 + 512 if c < 4 else r0 - 512
    o2 = _band_rows(img2, r0, 128)
    p2 = _band_rows(img2, pbase, 128)
    o1 = _band_rows(img1, r0, 128)
    p1 = _band_rows(img1, pbase, 128)
    A, Bm = _pack_pair(o2, p2, o1, p1, 128)

    CM = np.ones((128, 2), np.float32)
    CM[0:7, 0] = 0.0
    CM[121:128, 1] = 0.0

    return {
        "in1": A.astype(BF),
        "in2": Bm.astype(BF),
        "hs1": _host_halo_s1(img1, img2, c).astype(BF),
        "colmask": CM,
    }


# ---------------------------------------------------------------- bass build

_NC_CACHE = {}


def _build_nc():
    import concourse.bacc as bacc
    import concourse.mybir as mybir
    from concourse.tile import TileContext

    f32 = mybir.dt.float32
    bf16 = mybir.dt.bfloat16
    Alu = mybir.AluOpType
    Act = mybir.ActivationFunctionType

    nc = bacc.Bacc("TRN2")

    d_in1 = nc.declare_dram_parameter("in1", [128, WIN], bf16, isOutput=False)
    d_in2 = nc.declare_dram_parameter("in2", [128, WIN], bf16, isOutput=False)
    d_hs1 = nc.declare_dram_parameter("hs1", [6, WC], bf16, isOutput=False)
    d_cm = nc.declare_dram_parameter("colmask", [128, 2], f32, isOutput=False)
    out_part = nc.declare_dram_parameter("partials", [128, 3], f32,
                                         isOutput=True)

    g = [float(v) for v in G7]
    i_s0 = FOFF["s0"][1]
    i_f = FOFF["f"][1]
    i_x = FOFF["x"][1]
    i_x2 = FOFF["x2"][1]
    i_p = FOFF["p"][1]
    i_m = FOFF["m"][1]
    i_sv = FOFF["sv"][1]
    i_oq = FOFF["o1sq"][1]
    i_vx = FOFF["vx"][1]

    with TileContext(nc) as tc:
        with tc.tile_pool(name="pers", bufs=1) as P:
            def tile(tag, shp, dt=bf16):
                return P.tile(shp, dt, tag=tag, name=tag)

            V = nc.vector
            SC = nc.scalar
            Am, Aa = Alu.mult, Alu.add

            IN1 = tile("IN1", [128, WIN])
            IN2 = tile("IN2", [128, WIN])
            HS1 = tile("HS1", [6, WC])
            colmask = tile("colmask", [128, 2], f32)
            nc.sync.dma_start(out=IN1[:], in_=d_in1[:])
            nc.sync.dma_start(out=IN2[:], in_=d_in2[:])
            nc.sync.dma_start(out=HS1[:], in_=d_hs1[:])
            nc.sync.dma_start(out=colmask[:], in_=d_cm[:])

            # squared fields: one sub + one mul
            D = tile("D", [128, WIN])
            V.tensor_sub(D[:], IN1[:], IN2[:])
            SQ = tile("SQ", [128, WIN])
            V.tensor_mul(SQ[:], D[:], D[:])

            # Vimg folds: sv += 2*o1sq; sv halves += vx
            V.scalar_tensor_tensor(SQ[:, i_sv:i_sv + 1024],
                                   SQ[:, i_oq:i_oq + 1024], 2.0,
                                   SQ[:, i_sv:i_sv + 1024], Am, Aa)
            V.tensor_add(SQ[:, i_sv:i_sv + 512],
                         SQ[:, i_vx:i_vx + 512], SQ[:, i_sv:i_sv + 512])
            V.tensor_add(SQ[:, i_sv + 512:i_sv + 1024],
                         SQ[:, i_vx:i_vx + 512],
                         SQ[:, i_sv + 512:i_sv + 1024])

            # x-conv: 7 shifted taps over the conv region
            S1 = tile("S1", [128, WC])
            V.memset(S1[:], 0.0)
            V.tensor_scalar_mul(S1[:, 3:WC - 3], SQ[:, 3:WC - 3], g[3])
            for k in (0, 1, 2, 4, 5, 6):
                V.scalar_tensor_tensor(S1[:, 3:WC - 3], SQ[:, k:k + WC - 6],
                                       g[k], S1[:, 3:WC - 3], Am, Aa)

            # y-conv: partition-shifted copies (halo rows from HS1) + 7 taps
            shifts = {}
            for d in (1, 2, 3):
                tp = tile(f"Sp{d}", [128, WC])   # T[r] = S1[r+d]
                nc.sync.dma_start(out=tp[0:128 - d, :], in_=S1[d:128, :])
                nc.sync.dma_start(out=tp[128 - d:128, :],
                                  in_=HS1[3:3 + d, :])
                shifts[d] = tp
                tm = tile(f"Sm{d}", [128, WC])   # T[r] = S1[r-d]
                nc.sync.dma_start(out=tm[d:128, :], in_=S1[0:128 - d, :])
                nc.sync.dma_start(out=tm[0:d, :], in_=HS1[3 - d:3, :])
                shifts[-d] = tm

            D2 = tile("D2", [128, WC])
            V.tensor_scalar_mul(D2[:], S1[:], g[3])
            for d in (-3, -2, -1, 1, 2, 3):
                V.scalar_tensor_tensor(D2[:], shifts[d][:], g[3 + d],
                                       D2[:], Am, Aa)

            # vinv = -1/(0.25*blurV + eps)
            nv = tile("nv", [128, 1024], f32)
            nvinv = tile("nvinv", [128, 1024], f32)
            V.tensor_scalar(nv[:], D2[:, i_sv:i_sv + 1024], -0.25, -EPS,
                            Am, Aa)
            V.reciprocal_approx_fast(out=nvinv[:], in_=nv[:])

            # b = D2 * nvinv for the 4 map layers (s0, f, x->po|mo, pu|mu)
            B = tile("B", [128, 4, 1024])
            V.tensor_tensor(B[:, 0, :], D2[:, i_s0:i_s0 + 1024], nvinv[:], Am)
            V.tensor_tensor(B[:, 1, :], D2[:, i_f:i_f + 1024], nvinv[:], Am)
            V.tensor_tensor(B[:, 2, 0:512], D2[:, i_x:i_x + 512],
                            nvinv[:, 0:512], Am)
            V.tensor_tensor(B[:, 2, 512:1024], D2[:, i_x2:i_x2 + 512],
                            nvinv[:, 512:1024], Am)
            V.tensor_tensor(B[:, 3, 0:512], D2[:, i_p:i_p + 512],
                            nvinv[:, 0:512], Am)
            V.tensor_tensor(B[:, 3, 512:1024], D2[:, i_m:i_m + 512],
                            nvinv[:, 512:1024], Am)

            # e = exp(b); bmax = max over maps; num = 77*e0 + e1 + e2 + e3
            E = tile("E", [128, 4, 1024])
            SC.activation(E[:], B[:], Act.Exp)
            tmax = tile("tmax", [128, 2, 1024])
            V.tensor_tensor(tmax[:], B[:, 0:2, :], B[:, 2:4, :], Alu.max)
            bmax = tile("bmax", [128, 1024])
            V.tensor_tensor(bmax[:], tmax[:, 0, :], tmax[:, 1, :], Alu.max)
            n1 = tile("n1", [128, 1024])
            V.scalar_tensor_tensor(n1[:], E[:, 0, :], 77.0, E[:, 1, :],
                                   Am, Aa)
            n2 = tile("n2", [128, 1024])
            V.tensor_add(n2[:], E[:, 2, :], E[:, 3, :])
            num = tile("num", [128, 8, 128])
            V.tensor_add(num[:], n1[:].rearrange("p (a b) -> p a b", b=128),
                         n2[:].rearrange("p (a b) -> p a b", b=128))

            rden = tile("rden", [128, 8, 128])
            SC.activation(rden[:],
                          bmax[:].rearrange("p (a b) -> p a b", b=128),
                          Act.Exp, scale=-1.0)

            # column crop: zero global cols 0-6 and 1017-1023 in num
            V.tensor_scalar(num[:, 0, :], num[:, 0, :], colmask[:, 0:1], None,
                            Am)
            V.tensor_scalar(num[:, 7, :], num[:, 7, :], colmask[:, 1:2], None,
                            Am)

            # fused multiply + per-column reduce
            partials = tile("partials", [128, 3], f32)
            scr = tile("scr", [128, 8, 128])
            V.scalar_tensor_tensor(scr[:], num[:], 1.0, rden[:],
                                   Am, Am, accum_out=partials[:, 0:1])
            V.scalar_tensor_tensor(scr[:, :, 0:7], num[:, :, 0:7], 1.0,
                                   rden[:, :, 0:7], Am, Am,
                                   accum_out=partials[:, 1:2])
            V.scalar_tensor_tensor(scr[:, :, 122:128], num[:, :, 122:128],
                                   1.0, rden[:, :, 122:128], Am, Am,
                                   accum_out=partials[:, 2:3])

            nc.sync.dma_start(out=out_part[:], in_=partials[:])

    nc.finalize()
    return nc


def _get_nc():
    if "nc" not in _NC_CACHE:
        _NC_CACHE["nc"] = _build_nc()
    return _NC_CACHE["nc"]


# ---------------------------------------------------------------- entry point

def kernel(image1, image2, _trace=False):
    from concourse.bass_utils import run_bass_kernel_spmd

    img1 = np.asarray(image1, np.float32)[0, 0]
    img2 = np.asarray(image2, np.float32)[0, 0]
    in_maps = [_core_inputs(img1, img2, c) for c in range(8)]

    nc = _get_nc()
    res = run_bass_kernel_spmd(nc, in_maps, list(range(8)), trace=_trace)

    total = 0.0
    for c, r in enumerate(res.results):
        p = np.asarray(r["partials"], np.float64)
        s = p[:, 0].sum()
        if c == 0:
            s -= p[:, 1].sum()
        if c == 7:
            s -= p[:, 2].sum()
        total += s
    return np.float32(total / NORM)
